# revision 6
# baseline (speedup 1.0000x reference)
"""Trainium2 Bass kernel for fused multi-head attention block
(qkv projection + RoPE + GQA causal attention + dense out projection),
tensor-parallel over 8 NeuronCores.

Sharding: q/k/v heads split across cores (4 q heads + 1 kv head per core),
dense input dim sharded correspondingly; attention outputs are AllGathered
(bf16) and each core computes a 512-column shard of the dense output.

Device-side layout notes:
- qkv activations are fed pre-transposed [E, B*S] so the projection
  contracts E on partitions; q/k come out as [head_dim, tokens] tiles
  (head_dim on partitions), which feeds the scoresT formulation directly.
- RoPE: w_qkv rows for q/k are host-permuted (even d first, odd d second)
  so the rotation becomes two full-tile multiplies + one partition-half
  swap (done by DMA) + one add.
- attention is computed transposed: scoresT[ki, qi] = k^T q, softmax sums
  over ki obtained with a ones-vector matmul, exp on the scalar engine
  (no max subtraction: scores are ~N(0,1) here so exp is safe in f32).
- v is projected in [tokens, head_dim] layout so PV matmul needs no
  transposes anywhere.
"""

import sys

sys.path.insert(0, "/opt/trn_rl_repo")

import ml_dtypes
import numpy as np

import concourse.bass as bass
import concourse.mybir as mybir
import concourse.tile as tile
from concourse import bacc
from concourse.bass_utils import run_bass_kernel_spmd

F32 = mybir.dt.float32
BF16 = mybir.dt.bfloat16
BF16NP = ml_dtypes.bfloat16

# problem constants (hardcoded per harness contract)
B, S, E = 2, 2048, 4096
NH, KVH, D = 32, 8, 128
ROPE_THETA = 10000.0
NCORES = 8
HLOC = NH // NCORES  # 4 q heads per core
T = B * S  # 4096 tokens
KC = E // 128  # 32 contraction chunks
NT = T // 512  # 8 token blocks of 512
QKM = HLOC + 1  # 5 projection feature tiles (4 q + 1 k)
ESH = NH * D // NCORES  # 512 output columns per core
SCALE = 1.0 / float(np.sqrt(D))

_CACHED_NC = None


def build_nc():
    nc = bacc.Bacc(None, num_devices=NCORES)

    x_t = nc.dram_tensor("x_t", [E, T], BF16, kind="ExternalInput")
    w_qk = nc.dram_tensor("w_qk", [E, QKM * 128], BF16, kind="ExternalInput")
    w_v = nc.dram_tensor("w_v", [E, D], BF16, kind="ExternalInput")
    w_d = nc.dram_tensor("w_d", [NH * D, ESH], BF16, kind="ExternalInput")
    cosb = nc.dram_tensor("cosb", [128, T], BF16, kind="ExternalInput")
    sinbs = nc.dram_tensor("sinbs", [128, T], BF16, kind="ExternalInput")
    out = nc.dram_tensor("out", [T, ESH], F32, kind="ExternalOutput")

    attn_loc = [
        nc.dram_tensor(f"attn_loc{b}", [HLOC * D, S], BF16) for b in range(B)
    ]
    attn_g = [
        nc.dram_tensor(f"attn_g{b}", [NH * D, S], BF16, addr_space="Shared")
        for b in range(B)
    ]
    rstage = [nc.dram_tensor(f"rstage{i}", [S], F32) for i in range(B * HLOC)]

    umask_np = np.triu(np.ones((128, 128), np.float32)).astype(BF16NP)
    umask_d = nc.inline_tensor(umask_np, name="umask_const")
    ones_d = nc.inline_tensor(np.ones((128, 1), BF16NP), name="ones_const")

    x_t3 = x_t.rearrange("(k p) t -> p k t", p=128)
    w_qk3 = w_qk.rearrange("(k p) m -> p k m", p=128)
    w_v3 = w_v.rearrange("(k p) m -> p k m", p=128)
    w_d3 = w_d.rearrange("(k p) m -> p k m", p=128)

    with tile.TileContext(nc) as tc:
        with (
            tc.tile_pool(name="persist", bufs=1) as persist,
            tc.tile_pool(name="cst", bufs=1) as cst,
        ):
            # persistent across phases
            qk_sb = persist.tile([128, QKM, T], BF16)  # q/k projections, later rope'd
            v_sb = persist.tile([128, T // 128, D], BF16)  # v in [tok, dv] tiles
            ones_sb = cst.tile([128, 1], BF16)
            umask_sb = cst.tile([128, 128], BF16)
            nc.sync.dma_start(out=ones_sb, in_=ones_d[:, :])
            nc.sync.dma_start(out=umask_sb, in_=umask_d[:, :])

            # ---------------- Phase 1: fused qkv projection ----------------
            with (
                tc.tile_pool(name="wqk", bufs=1) as wqk,
                tc.tile_pool(name="xin", bufs=2) as xin,
                tc.tile_pool(name="ps_qk", bufs=6, space="PSUM") as ps_qk,
                tc.tile_pool(name="ps_v", bufs=2, space="PSUM") as ps_v,
            ):
                w_qk_sb = wqk.tile([128, KC, QKM * 128], BF16)
                w_v_sb = wqk.tile([128, KC, D], BF16)
                nc.sync.dma_start(out=w_qk_sb, in_=w_qk3)
                nc.sync.dma_start(out=w_v_sb, in_=w_v3)

                for n in range(NT):  # 512-token blocks
                    x_blk = xin.tile([128, KC, 512], BF16)
                    nc.sync.dma_start(
                        out=x_blk, in_=x_t3[:, :, n * 512 : (n + 1) * 512]
                    )
                    pq = [
                        ps_qk.tile([128, 512], F32, tag="qk", name=f"pq{m}")
                        for m in range(QKM)
                    ]
                    for k in range(KC):
                        st, sp = k == 0, k == KC - 1
                        for m in range(QKM):
                            nc.tensor.matmul(
                                pq[m],
                                lhsT=w_qk_sb[:, k, m * 128 : (m + 1) * 128],
                                rhs=x_blk[:, k, :],
                                start=st,
                                stop=sp,
                            )
                    for m in range(QKM):
                        nc.scalar.copy(
                            out=qk_sb[:, m, n * 512 : (n + 1) * 512], in_=pq[m]
                        )
                    # v tiles each get their own PSUM bank: start=True clears
                    # has_written for the whole bank, so regions of one bank
                    # cannot host interleaved accumulation groups
                    for tt in range(4):
                        pv = ps_v.tile([128, 128], F32, tag="v", name=f"pv{tt}")
                        for k in range(KC):
                            nc.tensor.matmul(
                                pv,
                                lhsT=x_blk[:, k, tt * 128 : (tt + 1) * 128],
                                rhs=w_v_sb[:, k, :],
                                start=(k == 0),
                                stop=(k == KC - 1),
                            )
                        nc.vector.tensor_copy(
                            out=v_sb[:, n * 4 + tt, :], in_=pv
                        )

            # w_d load: traced here so the DMA can overlap attention
            with tc.tile_pool(name="wd", bufs=1) as wd:
                w_d_sb = wd.tile([128, KC, ESH], BF16)
                nc.sync.dma_start(out=w_d_sb, in_=w_d3)

                # ---------------- Phase 1.5: RoPE on q and k ----------------
                with tc.tile_pool(name="trig", bufs=1) as trig:
                    cos_sb = trig.tile([128, T], BF16)
                    sin_sb = trig.tile([128, T], BF16)
                    nc.sync.dma_start(out=cos_sb, in_=cosb[:, :])
                    nc.sync.dma_start(out=sin_sb, in_=sinbs[:, :])
                    with tc.tile_pool(name="ropetmp", bufs=2) as ropetmp:
                        for m in range(QKM):
                            ta = ropetmp.tile([128, T], BF16, tag="ta")
                            tb = ropetmp.tile([128, T], BF16, tag="tb")
                            tbs = ropetmp.tile([128, T], BF16, tag="tbs")
                            raw = qk_sb[:, m, :]
                            nc.vector.tensor_mul(ta, raw, cos_sb)
                            # sin_sb holds [sin; -sin] so after the half swap the
                            # signs line up for a single add
                            nc.vector.tensor_mul(tb, raw, sin_sb)
                            nc.sync.dma_start(out=tbs[0:64, :], in_=tb[64:128, :])
                            nc.sync.dma_start(out=tbs[64:128, :], in_=tb[0:64, :])
                            nc.vector.tensor_add(qk_sb[:, m, :], ta, tbs)

                # ---------------- Phase 2: causal GQA attention ----------------
                with (
                    tc.tile_pool(name="probs", bufs=4) as probs,
                    tc.tile_pool(name="attnsb", bufs=2) as attnsb,
                    tc.tile_pool(name="norm", bufs=2) as norm,
                    tc.tile_pool(name="recb", bufs=2) as recb,
                    tc.tile_pool(name="ps_s", bufs=2, space="PSUM") as ps_s,
                    tc.tile_pool(name="ps_a", bufs=2, space="PSUM") as ps_a,
                    tc.tile_pool(name="ps_c", bufs=2, space="PSUM") as ps_c,
                    tc.tile_pool(name="ps_o", bufs=2, space="PSUM") as ps_o,
                    tc.tile_pool(name="gin", bufs=2) as gin,
                    tc.tile_pool(name="osb", bufs=3) as osb,
                ):
                    for b in range(B):
                        kT = qk_sb[:, HLOC, b * S : (b + 1) * S]
                        for h in range(HLOC):
                            qT = qk_sb[:, h, b * S : (b + 1) * S]
                            au = attnsb.tile([128, S], BF16, tag="au")
                            cs = norm.tile([1, S], F32, tag="cs")
                            for qb in range(4):
                                a_ps = ps_a.tile([128, 512], F32)
                                c_ps = ps_c.tile([1, 512], F32)
                                nkt = 4 * (qb + 1)
                                for kt in range(nkt):
                                    s_ps = ps_s.tile([128, 512], F32)
                                    nc.tensor.matmul(
                                        s_ps,
                                        lhsT=kT[:, kt * 128 : (kt + 1) * 128],
                                        rhs=qT[:, qb * 512 : (qb + 1) * 512],
                                        start=True,
                                        stop=True,
                                    )
                                    pT = probs.tile([128, 512], BF16)
                                    lo = kt * 128 - qb * 512
                                    if lo > 0:
                                        nc.gpsimd.memset(pT[:, 0:lo], 0.0)
                                    lo = max(lo, 0)
                                    nc.scalar.activation(
                                        out=pT[:, lo:512],
                                        in_=s_ps[:, lo:512],
                                        func=mybir.ActivationFunctionType.Exp,
                                        scale=SCALE,
                                    )
                                    if kt * 128 >= qb * 512:
                                        # diagonal 128x128 subtile: causal mask
                                        nc.vector.tensor_mul(
                                            pT[:, lo : lo + 128],
                                            pT[:, lo : lo + 128],
                                            umask_sb,
                                        )
                                    st, sp = kt == 0, kt == nkt - 1
                                    nc.tensor.matmul(
                                        a_ps,
                                        lhsT=v_sb[:, b * 16 + kt, :],
                                        rhs=pT,
                                        start=st,
                                        stop=sp,
                                    )
                                    nc.tensor.matmul(
                                        c_ps,
                                        lhsT=ones_sb,
                                        rhs=pT,
                                        start=st,
                                        stop=sp,
                                    )
                                nc.vector.tensor_copy(
                                    out=au[:, qb * 512 : (qb + 1) * 512], in_=a_ps
                                )
                                nc.scalar.copy(
                                    out=cs[0:1, qb * 512 : (qb + 1) * 512], in_=c_ps
                                )
                            # softmax denominator -> reciprocal -> broadcast
                            rec = norm.tile([1, S], F32, tag="rec")
                            nc.vector.reciprocal_approx_fast(out=rec, in_=cs)
                            sid = b * HLOC + h
                            nc.sync.dma_start(out=rstage[sid][:], in_=rec)
                            rb = recb.tile([128, S], F32)
                            rsrc = rstage[sid].ap()
                            nc.sync.dma_start(
                                out=rb,
                                in_=bass.AP(
                                    tensor=rsrc.tensor,
                                    offset=rsrc.offset,
                                    ap=[[0, 128]] + rsrc.ap,
                                ),
                            )
                            nc.vector.tensor_mul(au, au, rb)
                            nc.sync.dma_start(
                                out=attn_loc[b][h * 128 : (h + 1) * 128, :], in_=au
                            )
                        nc.gpsimd.collective_compute(
                            "AllGather",
                            mybir.AluOpType.bypass,
                            replica_groups=[list(range(NCORES))],
                            ins=[attn_loc[b].ap().opt()],
                            outs=[attn_g[b].ap().opt()],
                        )

                    # ---------------- Phase 3: dense output shard ----------------
                    for b in range(B):
                        g3 = attn_g[b].rearrange("(k p) s -> p k s", p=128)
                        for tg in range(8):  # 256-token groups
                            gt = gin.tile([128, KC, 256], BF16)
                            nc.sync.dma_start(
                                out=gt, in_=g3[:, :, tg * 256 : (tg + 1) * 256]
                            )
                            for tt in range(2):  # 128-token tiles
                                o_ps = ps_o.tile([128, ESH], F32)
                                for k in range(KC):
                                    nc.tensor.matmul(
                                        o_ps,
                                        lhsT=gt[:, k, tt * 128 : (tt + 1) * 128],
                                        rhs=w_d_sb[:, k, :],
                                        start=(k == 0),
                                        stop=(k == KC - 1),
                                    )
                                o_sb = osb.tile([128, ESH], F32)
                                if tt % 2 == 0:
                                    nc.vector.tensor_copy(out=o_sb, in_=o_ps)
                                else:
                                    nc.scalar.copy(out=o_sb, in_=o_ps)
                                row = (b * 16 + tg * 2 + tt) * 128
                                nc.sync.dma_start(
                                    out=out[row : row + 128, :], in_=o_sb
                                )

    nc.compile()
    return nc


def _prep_inputs(qkv, position_ids, w_qkv, w_dense):
    """Host-side sharding/layout prep. Returns per-core input maps."""
    qkv = np.asarray(qkv, dtype=np.float32)
    w_qkv = np.asarray(w_qkv, dtype=np.float32)
    w_dense = np.asarray(w_dense, dtype=np.float32)
    pos = np.asarray(position_ids).reshape(-1).astype(np.float64)  # [S]

    x_t = np.ascontiguousarray(qkv.reshape(T, E).T).astype(BF16NP)  # [E, T]

    # rope tables: rows 0:64 = freq f for even-d slots, rows 64:128 duplicated
    inv_freq = 1.0 / (ROPE_THETA ** (np.arange(0, D, 2, dtype=np.float64) / D))
    ang = inv_freq[:, None] * pos[None, :]  # [64, S]
    cos_h = np.cos(ang).astype(np.float32)
    sin_h = np.sin(ang).astype(np.float32)
    cosb = np.tile(np.concatenate([cos_h, cos_h], axis=0), (1, B)).astype(BF16NP)
    # sign-folded: [sin; -sin] so rope = ta + swap(tb)
    sinbs = np.tile(np.concatenate([sin_h, -sin_h], axis=0), (1, B)).astype(BF16NP)

    perm = np.concatenate([np.arange(0, D, 2), np.arange(1, D, 2)])  # even, odd

    wq = w_qkv[: NH * D].reshape(NH, D, E)[:, perm, :]  # [NH, D, E] permuted
    wk = w_qkv[NH * D : NH * D + KVH * D].reshape(KVH, D, E)[:, perm, :]
    wv = w_qkv[NH * D + KVH * D :].reshape(KVH, D, E)

    in_maps = []
    for c in range(NCORES):
        wqk_rows = np.concatenate(
            [wq[4 * c + h] for h in range(HLOC)] + [wk[c]], axis=0
        )  # [640, E]
        w_qk_t = np.ascontiguousarray(wqk_rows.T).astype(BF16NP)  # [E, 640]
        w_v_t = np.ascontiguousarray(wv[c].T).astype(BF16NP)  # [E, 128]
        w_d_t = np.ascontiguousarray(
            w_dense[c * ESH : (c + 1) * ESH, :].T
        ).astype(BF16NP)  # [4096, 512]
        in_maps.append(
            {
                "x_t": x_t,
                "w_qk": w_qk_t,
                "w_v": w_v_t,
                "w_d": w_d_t,
                "cosb": cosb,
                "sinbs": sinbs,
            }
        )
    return in_maps


def _run(inputs, trace=False, **kw):
    global _CACHED_NC
    if _CACHED_NC is None:
        _CACHED_NC = build_nc()
    in_maps = _prep_inputs(
        inputs["qkv"], inputs["position_ids"], inputs["w_qkv"], inputs["w_dense"]
    )
    res = run_bass_kernel_spmd(
        _CACHED_NC, in_maps, core_ids=list(range(NCORES)), trace=trace, **kw
    )
    full = np.empty((T, NH * D), dtype=np.float32)
    for c in range(NCORES):
        full[:, c * ESH : (c + 1) * ESH] = res.results[c]["out"]
    return full.reshape(B, S, NH * D), res


def kernel(**inputs) -> np.ndarray:
    out, _ = _run(inputs, trace=False)
    return out


def time_steady(inputs, iters=20):
    """Steady-state on-device timing: inputs pre-placed on the 8 cores,
    jitted shard_map executable re-run `iters` times. Returns per-call
    wall seconds plus the outputs (for correctness cross-check)."""
    import time as _time

    import jax
    from jax.experimental.shard_map import shard_map
    from jax.sharding import Mesh, NamedSharding, PartitionSpec

    from concourse.bass2jax import (
        _bass_exec_p,
        install_neuronx_cc_hook,
        partition_id_tensor,
    )

    global _CACHED_NC
    if _CACHED_NC is None:
        _CACHED_NC = build_nc()
    nc = _CACHED_NC
    install_neuronx_cc_hook()
    in_maps = _prep_inputs(
        inputs["qkv"], inputs["position_ids"], inputs["w_qkv"], inputs["w_dense"]
    )

    partition_name = nc.partition_id_tensor.name if nc.partition_id_tensor else None
    in_names, out_names, out_avals = [], [], []
    for alloc in nc.m.functions[0].allocations:
        if not isinstance(alloc, mybir.MemoryLocationSet):
            continue
        name = alloc.memorylocations[0].name
        if alloc.kind == "ExternalInput":
            if name != partition_name:
                in_names.append(name)
        elif alloc.kind == "ExternalOutput":
            out_names.append(name)
            out_avals.append(
                jax.core.ShapedArray(
                    tuple(alloc.tensor_shape), mybir.dt.np(alloc.dtype)
                )
            )
    n_params = len(in_names)
    all_in = in_names + out_names + ([partition_name] if partition_name else [])

    def _body(*args):
        operands = list(args)
        if partition_name is not None:
            operands.append(partition_id_tensor())
        return tuple(
            _bass_exec_p.bind(
                *operands,
                out_avals=tuple(out_avals),
                in_names=tuple(all_in),
                out_names=tuple(out_names),
                lowering_input_output_aliases=(),
                sim_require_finite=True,
                sim_require_nnan=True,
                nc=nc,
            )
        )

    devices = jax.devices()[:NCORES]
    mesh = Mesh(np.asarray(devices), ("core",))
    nio = n_params + len(out_names)
    f = jax.jit(
        shard_map(
            _body,
            mesh=mesh,
            in_specs=(PartitionSpec("core"),) * nio,
            out_specs=(PartitionSpec("core"),) * len(out_names),
            check_rep=False,
        ),
        keep_unused=True,
    )
    sh = NamedSharding(mesh, PartitionSpec("core"))
    concat_in = [
        jax.device_put(
            np.concatenate([np.asarray(in_maps[c][nm]) for c in range(NCORES)]), sh
        )
        for nm in in_names
    ]
    concat_zeros = [
        jax.device_put(np.zeros((NCORES * a.shape[0], *a.shape[1:]), a.dtype), sh)
        for a in out_avals
    ]
    outs = f(*concat_in, *concat_zeros)
    jax.block_until_ready(outs)
    times = []
    for _ in range(iters):
        t0 = _time.perf_counter()
        outs = f(*concat_in, *concat_zeros)
        jax.block_until_ready(outs)
        times.append(_time.perf_counter() - t0)

    res = {}
    for i, nm in enumerate(out_names):
        arr = np.asarray(outs[i]).reshape(NCORES, *out_avals[i].shape)
        res[nm] = arr
    return times, res


if __name__ == "__main__":
    rng = np.random.default_rng(0)
    ins = {
        "qkv": rng.standard_normal((B, S, E), dtype=np.float32),
        "position_ids": np.arange(S, dtype=np.int32)[None, :],
        "w_qkv": (rng.standard_normal((6144, E), dtype=np.float32) * E**-0.5),
        "w_dense": (rng.standard_normal((E, NH * D), dtype=np.float32) * E**-0.5),
    }
    out = kernel(**ins)
    print("ran ok", out.shape, out.dtype, np.abs(out).max())


# revision 12
# speedup vs baseline: 1.0896x; 1.0896x over previous
"""Trainium2 Bass kernel for fused multi-head attention block
(qkv projection + RoPE + GQA causal attention + dense out projection),
tensor-parallel over 8 NeuronCores.

Sharding: q/k/v heads split across cores (4 q heads + 1 kv head per core),
dense input dim sharded correspondingly; attention outputs are AllGathered
(bf16) and each core computes a 512-column shard of the dense output.

Device-side layout notes:
- qkv activations are fed pre-transposed [E, B*S] so the projection
  contracts E on partitions; q/k come out as [head_dim, tokens] tiles
  (head_dim on partitions), which feeds the scoresT formulation directly.
- RoPE: w_qkv rows for q/k are host-permuted (even d first, odd d second)
  so the rotation becomes two full-tile multiplies + one partition-half
  swap (done by DMA) + one add.
- attention is computed transposed: scoresT[ki, qi] = k^T q, softmax sums
  over ki obtained with a ones-vector matmul, exp on the scalar engine
  (no max subtraction: scores are ~N(0,1) here so exp is safe in f32).
- v is projected in [tokens, head_dim] layout so PV matmul needs no
  transposes anywhere.
"""

import sys

sys.path.insert(0, "/opt/trn_rl_repo")

import ml_dtypes
import numpy as np

import concourse.bass as bass
import concourse.mybir as mybir
import concourse.tile as tile
from concourse import bacc
from concourse.bass_utils import run_bass_kernel_spmd

F32 = mybir.dt.float32
BF16 = mybir.dt.bfloat16
BF16NP = ml_dtypes.bfloat16

# problem constants (hardcoded per harness contract)
B, S, E = 2, 2048, 4096
NH, KVH, D = 32, 8, 128
ROPE_THETA = 10000.0
NCORES = 8
HLOC = NH // NCORES  # 4 q heads per core
T = B * S  # 4096 tokens
KC = E // 128  # 32 contraction chunks
NT = T // 512  # 8 token blocks of 512
QKM = HLOC + 1  # 5 projection feature tiles (4 q + 1 k)
ESH = NH * D // NCORES  # 512 output columns per core
SCALE = 1.0 / float(np.sqrt(D))

_CACHED_NC = None


def build_nc():
    nc = bacc.Bacc(None, num_devices=NCORES)

    x_t = nc.dram_tensor("x_t", [E, T], BF16, kind="ExternalInput")
    w_qk = nc.dram_tensor("w_qk", [E, QKM * 128], BF16, kind="ExternalInput")
    w_v = nc.dram_tensor("w_v", [E, D], BF16, kind="ExternalInput")
    w_d = nc.dram_tensor("w_d", [NH * D, ESH], BF16, kind="ExternalInput")
    cosb = nc.dram_tensor("cosb", [128, T], BF16, kind="ExternalInput")
    sinbs = nc.dram_tensor("sinbs", [128, T], BF16, kind="ExternalInput")
    out = nc.dram_tensor("out", [T, ESH], F32, kind="ExternalOutput")

    attn_loc = nc.dram_tensor("attn_loc", [HLOC * D, T], BF16)
    attn_g = nc.dram_tensor("attn_g", [NH * D, T], BF16, addr_space="Shared")

    umask_np = np.triu(np.ones((128, 128), np.float32)).astype(BF16NP)
    umask_d = nc.inline_tensor(umask_np, name="umask_const")
    ones_d = nc.inline_tensor(np.ones((128, 1), BF16NP), name="ones_const")
    ones_row_d = nc.inline_tensor(np.ones((1, 128), np.float32), name="ones_row_const")

    x_t3 = x_t.rearrange("(k p) t -> p k t", p=128)
    w_qk3 = w_qk.rearrange("(k p) m -> p k m", p=128)
    w_v3 = w_v.rearrange("(k p) m -> p k m", p=128)
    w_d3 = w_d.rearrange("(k p) m -> p k m", p=128)

    with tile.TileContext(nc) as tc:
        with (
            tc.tile_pool(name="persist", bufs=1) as persist,
            tc.tile_pool(name="cst", bufs=1) as cst,
        ):
            qk_sb = persist.tile([128, QKM, T], BF16)  # q/k proj, rope'd in place
            v_sb = persist.tile([128, T // 128, D], BF16)  # v in [tok, dv] tiles
            ones_sb = cst.tile([128, 1], BF16)
            ones_row_sb = cst.tile([1, 128], F32)
            umask_sb = cst.tile([128, 128], BF16)
            nc.sync.dma_start(out=ones_sb, in_=ones_d[:, :])
            nc.sync.dma_start(out=ones_row_sb, in_=ones_row_d[:, :])
            nc.sync.dma_start(out=umask_sb, in_=umask_d[:, :])

            # rope helper: rotate one [128, S] half of one projection tile
            def rope_half(ropetmp, m, bb):
                cols = slice(bb * S, (bb + 1) * S)
                ta = ropetmp.tile([128, S], BF16, tag="ta", name="ta")
                tb = ropetmp.tile([128, S], BF16, tag="tb", name="tb")
                tbs = ropetmp.tile([128, S], BF16, tag="tbs", name="tbs")
                raw = qk_sb[:, m, cols]
                nc.vector.tensor_mul(ta, raw, cos_sb[:, cols])
                # sin_sb holds [sin; -sin]; after the half swap the signs
                # line up for a single add
                nc.vector.tensor_mul(tb, raw, sin_sb[:, cols])
                nc.sync.dma_start(out=tbs[0:64, :], in_=tb[64:128, :])
                nc.sync.dma_start(out=tbs[64:128, :], in_=tb[0:64, :])
                nc.vector.tensor_add(qk_sb[:, m, cols], ta, tbs)

            with (
                tc.tile_pool(name="trig", bufs=1) as trig,
                tc.tile_pool(name="ropetmp", bufs=2) as ropetmp,
            ):
                cos_sb = trig.tile([128, T], BF16)
                sin_sb = trig.tile([128, T], BF16)
                nc.sync.dma_start(out=cos_sb, in_=cosb[:, :])
                nc.sync.dma_start(out=sin_sb, in_=sinbs[:, :])

                # ---------------- Phase 1: fused qkv projection ----------------
                with (
                    tc.tile_pool(name="wqk", bufs=1) as wqk,
                    tc.tile_pool(name="xin", bufs=2) as xin,
                    tc.tile_pool(name="ps_qk", bufs=6, space="PSUM") as ps_qk,
                    tc.tile_pool(name="ps_v", bufs=2, space="PSUM") as ps_v,
                ):
                    w_qk_sb = wqk.tile([128, KC, QKM * 128], BF16)
                    w_v_sb = wqk.tile([128, KC, D], BF16)
                    for ks in range(4):
                        nc.sync.dma_start(
                            out=w_qk_sb[:, ks * 8 : (ks + 1) * 8, :],
                            in_=w_qk3[:, ks * 8 : (ks + 1) * 8, :],
                        )
                    nc.sync.dma_start(out=w_v_sb, in_=w_v3)

                    for n in range(NT):  # 512-token blocks
                        x_blk = xin.tile([128, KC, 512], BF16)
                        if n == 0:
                            for ks in range(2):
                                nc.sync.dma_start(
                                    out=x_blk[:, ks * 16 : (ks + 1) * 16, :],
                                    in_=x_t3[:, ks * 16 : (ks + 1) * 16, 0:512],
                                )
                        else:
                            nc.sync.dma_start(
                                out=x_blk, in_=x_t3[:, :, n * 512 : (n + 1) * 512]
                            )
                        pq = [
                            ps_qk.tile([128, 512], F32, tag="qk", name=f"pq{m}")
                            for m in range(QKM)
                        ]
                        for k in range(KC):
                            st, sp = k == 0, k == KC - 1
                            for m in range(QKM):
                                nc.tensor.matmul(
                                    pq[m],
                                    lhsT=w_qk_sb[:, k, m * 128 : (m + 1) * 128],
                                    rhs=x_blk[:, k, :],
                                    start=st,
                                    stop=sp,
                                )
                        for m in range(QKM):
                            nc.scalar.copy(
                                out=qk_sb[:, m, n * 512 : (n + 1) * 512], in_=pq[m]
                            )
                        # v tiles each get their own PSUM bank: start=True
                        # clears has_written for the whole bank, so one bank
                        # cannot host interleaved accumulation groups
                        for tt in range(4):
                            pv = ps_v.tile([128, 128], F32, tag="v", name=f"pv{tt}")
                            for k in range(KC):
                                nc.tensor.matmul(
                                    pv,
                                    lhsT=x_blk[:, k, tt * 128 : (tt + 1) * 128],
                                    rhs=w_v_sb[:, k, :],
                                    start=(k == 0),
                                    stop=(k == KC - 1),
                                )
                            nc.vector.tensor_copy(
                                out=v_sb[:, n * 4 + tt, :], in_=pv
                            )
                        if n == 3:
                            # batch-0 halves of q/k fully projected: rope them
                            # now so attention can start the moment phase 1's
                            # matmuls drain
                            for m in (QKM - 1, *range(QKM - 1)):
                                rope_half(ropetmp, m, 0)
                    for m in (QKM - 1, *range(QKM - 1)):
                        rope_half(ropetmp, m, 1)

            with tc.tile_pool(name="wd", bufs=1) as wd:
                w_d_sb = wd.tile([128, KC, ESH], BF16)
                nc.sync.dma_start(out=w_d_sb, in_=w_d3)

                # ---------------- Phase 2: causal GQA attention ----------------
                last_au_dma = None
                last_attn_mm = None
                with (
                    tc.tile_pool(name="probs", bufs=6) as probs,
                    tc.tile_pool(name="attnsb", bufs=3) as attnsb,
                    tc.tile_pool(name="norm", bufs=2) as norm,
                    tc.tile_pool(name="ps_s", bufs=3, space="PSUM") as ps_s,
                    tc.tile_pool(name="ps_a", bufs=2, space="PSUM") as ps_a,
                    tc.tile_pool(name="ps_c", bufs=1, space="PSUM") as ps_c,
                    tc.tile_pool(name="ps_o", bufs=2, space="PSUM") as ps_o,
                    tc.tile_pool(name="gin", bufs=2) as gin,
                    tc.tile_pool(name="osb", bufs=3) as osb,
                ):
                    for b in range(B):
                        kT = qk_sb[:, HLOC, b * S : (b + 1) * S]
                        for h in range(HLOC):
                            qT = qk_sb[:, h, b * S : (b + 1) * S]
                            for qb in range(4):
                                a_ps = ps_a.tile([128, 512], F32, tag="a", name="a_ps")
                                c_ps = ps_c.tile([1, 512], F32, tag="c", name="c_ps")
                                nkt = 4 * (qb + 1)
                                for kt in range(nkt):
                                    s_ps = ps_s.tile([128, 512], F32)
                                    nc.tensor.matmul(
                                        s_ps,
                                        lhsT=kT[:, kt * 128 : (kt + 1) * 128],
                                        rhs=qT[:, qb * 512 : (qb + 1) * 512],
                                        start=True,
                                        stop=True,
                                    )
                                    pT = probs.tile([128, 512], BF16)
                                    lo = kt * 128 - qb * 512
                                    if lo > 0:
                                        nc.gpsimd.memset(pT[:, 0:lo], 0.0)
                                    lo = max(lo, 0)
                                    nc.scalar.activation(
                                        out=pT[:, lo:512],
                                        in_=s_ps[:, lo:512],
                                        func=mybir.ActivationFunctionType.Exp,
                                        scale=SCALE,
                                    )
                                    if kt * 128 >= qb * 512:
                                        # diagonal 128x128 subtile: causal mask
                                        nc.vector.tensor_mul(
                                            pT[:, lo : lo + 128],
                                            pT[:, lo : lo + 128],
                                            umask_sb,
                                        )
                                    st, sp = kt == 0, kt == nkt - 1
                                    nc.tensor.matmul(
                                        a_ps,
                                        lhsT=v_sb[:, b * 16 + kt, :],
                                        rhs=pT,
                                        start=st,
                                        stop=sp,
                                    )
                                    nc.tensor.matmul(
                                        c_ps,
                                        lhsT=ones_sb,
                                        rhs=pT,
                                        start=st,
                                        stop=sp,
                                    )
                                # denominator -> reciprocal -> PE row broadcast
                                rec = norm.tile([1, 512], F32, tag="rec")
                                nc.vector.reciprocal_approx_fast(out=rec, in_=c_ps)
                                rb = ps_a.tile([128, 512], F32, tag="a", name="rb")
                                last_attn_mm = nc.tensor.matmul(
                                    rb, lhsT=ones_row_sb, rhs=rec, start=True, stop=True
                                )
                                au = attnsb.tile([128, 512], BF16, tag="au")
                                nc.vector.tensor_copy(out=au, in_=a_ps)
                                nc.vector.tensor_mul(au, au, rb)
                                last_au_dma = nc.sync.dma_start(
                                    out=attn_loc[
                                        h * 128 : (h + 1) * 128,
                                        b * S + qb * 512 : b * S + (qb + 1) * 512,
                                    ],
                                    in_=au,
                                )
                    nc.gpsimd.collective_compute(
                        "AllGather",
                        mybir.AluOpType.bypass,
                        replica_groups=[list(range(NCORES))],
                        ins=[attn_loc.ap().opt()],
                        outs=[attn_g.ap().opt()],
                    )

                    # ---------------- Phase 3: dense output shard ----------------
                    g3 = attn_g.rearrange("(k p) s -> p k s", p=128)
                    first_gt_dma = None
                    first_dense_mm = None
                    for tg in range(16):  # 256-token groups
                        gt = gin.tile([128, KC, 256], BF16)
                        d = nc.sync.dma_start(
                            out=gt, in_=g3[:, :, tg * 256 : (tg + 1) * 256]
                        )
                        if first_gt_dma is None:
                            first_gt_dma = d
                        for tt in range(2):  # 128-token tiles
                            o_ps = ps_o.tile([128, ESH], F32)
                            for k in range(KC):
                                mm = nc.tensor.matmul(
                                    o_ps,
                                    lhsT=gt[:, k, tt * 128 : (tt + 1) * 128],
                                    rhs=w_d_sb[:, k, :],
                                    start=(k == 0),
                                    stop=(k == KC - 1),
                                )
                                if first_dense_mm is None:
                                    first_dense_mm = mm
                            o_sb = osb.tile([128, ESH], F32)
                            if tt % 2 == 0:
                                nc.vector.tensor_copy(out=o_sb, in_=o_ps)
                            else:
                                nc.scalar.copy(out=o_sb, in_=o_ps)
                            row = (tg * 2 + tt) * 128
                            nc.sync.dma_start(
                                out=out[row : row + 128, :], in_=o_sb
                            )
                    # ordering-only edges: keep every dense instruction after
                    # the attention tail in the per-engine streams, so the
                    # in-order engines never head-of-line block attention (and
                    # the collective issue) behind AG-gated dense work
                    tile.add_dep_helper(
                        first_gt_dma.ins,
                        last_au_dma.ins,
                        sync=False,
                        reason="order dense DMAs after attention output DMAs",
                    )
                    tile.add_dep_helper(
                        first_dense_mm.ins,
                        last_attn_mm.ins,
                        sync=False,
                        reason="order dense matmuls after attention matmuls",
                    )

    nc.compile()
    return nc


def _prep_inputs(qkv, position_ids, w_qkv, w_dense):
    """Host-side sharding/layout prep. Returns per-core input maps."""
    qkv = np.asarray(qkv, dtype=np.float32)
    w_qkv = np.asarray(w_qkv, dtype=np.float32)
    w_dense = np.asarray(w_dense, dtype=np.float32)
    pos = np.asarray(position_ids).reshape(-1).astype(np.float64)  # [S]

    x_t = np.ascontiguousarray(qkv.reshape(T, E).T).astype(BF16NP)  # [E, T]

    # rope tables: rows 0:64 = freq f for even-d slots, rows 64:128 duplicated
    inv_freq = 1.0 / (ROPE_THETA ** (np.arange(0, D, 2, dtype=np.float64) / D))
    ang = inv_freq[:, None] * pos[None, :]  # [64, S]
    cos_h = np.cos(ang).astype(np.float32)
    sin_h = np.sin(ang).astype(np.float32)
    cosb = np.tile(np.concatenate([cos_h, cos_h], axis=0), (1, B)).astype(BF16NP)
    # sign-folded: [sin; -sin] so rope = ta + swap(tb)
    sinbs = np.tile(np.concatenate([sin_h, -sin_h], axis=0), (1, B)).astype(BF16NP)

    perm = np.concatenate([np.arange(0, D, 2), np.arange(1, D, 2)])  # even, odd

    wq = w_qkv[: NH * D].reshape(NH, D, E)[:, perm, :]  # [NH, D, E] permuted
    wk = w_qkv[NH * D : NH * D + KVH * D].reshape(KVH, D, E)[:, perm, :]
    wv = w_qkv[NH * D + KVH * D :].reshape(KVH, D, E)

    in_maps = []
    for c in range(NCORES):
        wqk_rows = np.concatenate(
            [wq[4 * c + h] for h in range(HLOC)] + [wk[c]], axis=0
        )  # [640, E]
        w_qk_t = np.ascontiguousarray(wqk_rows.T).astype(BF16NP)  # [E, 640]
        w_v_t = np.ascontiguousarray(wv[c].T).astype(BF16NP)  # [E, 128]
        w_d_t = np.ascontiguousarray(
            w_dense[c * ESH : (c + 1) * ESH, :].T
        ).astype(BF16NP)  # [4096, 512]
        in_maps.append(
            {
                "x_t": x_t,
                "w_qk": w_qk_t,
                "w_v": w_v_t,
                "w_d": w_d_t,
                "cosb": cosb,
                "sinbs": sinbs,
            }
        )
    return in_maps


def _run(inputs, trace=False, **kw):
    global _CACHED_NC
    if _CACHED_NC is None:
        _CACHED_NC = build_nc()
    in_maps = _prep_inputs(
        inputs["qkv"], inputs["position_ids"], inputs["w_qkv"], inputs["w_dense"]
    )
    res = run_bass_kernel_spmd(
        _CACHED_NC, in_maps, core_ids=list(range(NCORES)), trace=trace, **kw
    )
    full = np.empty((T, NH * D), dtype=np.float32)
    for c in range(NCORES):
        full[:, c * ESH : (c + 1) * ESH] = res.results[c]["out"]
    return full.reshape(B, S, NH * D), res


def kernel(**inputs) -> np.ndarray:
    out, _ = _run(inputs, trace=False)
    return out


def time_steady(inputs, iters=20):
    """Steady-state on-device timing: inputs pre-placed on the 8 cores,
    jitted shard_map executable re-run `iters` times. Returns per-call
    wall seconds plus the outputs (for correctness cross-check)."""
    import time as _time

    import jax
    from jax.experimental.shard_map import shard_map
    from jax.sharding import Mesh, NamedSharding, PartitionSpec

    from concourse.bass2jax import (
        _bass_exec_p,
        install_neuronx_cc_hook,
        partition_id_tensor,
    )

    global _CACHED_NC
    if _CACHED_NC is None:
        _CACHED_NC = build_nc()
    nc = _CACHED_NC
    install_neuronx_cc_hook()
    in_maps = _prep_inputs(
        inputs["qkv"], inputs["position_ids"], inputs["w_qkv"], inputs["w_dense"]
    )

    partition_name = nc.partition_id_tensor.name if nc.partition_id_tensor else None
    in_names, out_names, out_avals = [], [], []
    for alloc in nc.m.functions[0].allocations:
        if not isinstance(alloc, mybir.MemoryLocationSet):
            continue
        name = alloc.memorylocations[0].name
        if alloc.kind == "ExternalInput":
            if name != partition_name:
                in_names.append(name)
        elif alloc.kind == "ExternalOutput":
            out_names.append(name)
            out_avals.append(
                jax.core.ShapedArray(
                    tuple(alloc.tensor_shape), mybir.dt.np(alloc.dtype)
                )
            )
    n_params = len(in_names)
    all_in = in_names + out_names + ([partition_name] if partition_name else [])

    def _body(*args):
        operands = list(args)
        if partition_name is not None:
            operands.append(partition_id_tensor())
        return tuple(
            _bass_exec_p.bind(
                *operands,
                out_avals=tuple(out_avals),
                in_names=tuple(all_in),
                out_names=tuple(out_names),
                lowering_input_output_aliases=(),
                sim_require_finite=True,
                sim_require_nnan=True,
                nc=nc,
            )
        )

    devices = jax.devices()[:NCORES]
    mesh = Mesh(np.asarray(devices), ("core",))
    nio = n_params + len(out_names)
    f = jax.jit(
        shard_map(
            _body,
            mesh=mesh,
            in_specs=(PartitionSpec("core"),) * nio,
            out_specs=(PartitionSpec("core"),) * len(out_names),
            check_rep=False,
        ),
        keep_unused=True,
    )
    sh = NamedSharding(mesh, PartitionSpec("core"))
    concat_in = [
        jax.device_put(
            np.concatenate([np.asarray(in_maps[c][nm]) for c in range(NCORES)]), sh
        )
        for nm in in_names
    ]
    concat_zeros = [
        jax.device_put(np.zeros((NCORES * a.shape[0], *a.shape[1:]), a.dtype), sh)
        for a in out_avals
    ]
    outs = f(*concat_in, *concat_zeros)
    jax.block_until_ready(outs)
    times = []
    for _ in range(iters):
        t0 = _time.perf_counter()
        outs = f(*concat_in, *concat_zeros)
        jax.block_until_ready(outs)
        times.append(_time.perf_counter() - t0)

    res = {}
    for i, nm in enumerate(out_names):
        arr = np.asarray(outs[i]).reshape(NCORES, *out_avals[i].shape)
        res[nm] = arr
    return times, res


if __name__ == "__main__":
    rng = np.random.default_rng(0)
    ins = {
        "qkv": rng.standard_normal((B, S, E), dtype=np.float32),
        "position_ids": np.arange(S, dtype=np.int32)[None, :],
        "w_qkv": (rng.standard_normal((6144, E), dtype=np.float32) * E**-0.5),
        "w_dense": (rng.standard_normal((E, NH * D), dtype=np.float32) * E**-0.5),
    }
    out = kernel(**ins)
    print("ran ok", out.shape, out.dtype, np.abs(out).max())
